# revision 1
# baseline (speedup 1.0000x reference)
"""Causal attention (B=2, H=16, L=2048, D=64, fp32) on 8 trn2 NeuronCores.

Sharding: the 32 (batch, head) pairs are split 4-per-core (pure data/head
parallelism, no cross-core comms). Each core runs the same Bass/Tile program
on its own 4 heads.

Device algorithm (per head):
  - Scores are computed TRANSPOSED: S_T[k, q] = sum_d K[k,d] Q[q,d], via
    matmul(lhsT=kT[:, kb*128:+128], rhs=qT[:, q_chunk]) -> PSUM [128k, 512q].
    Inputs are float32r (FP22-reduced fp32): full PE rate for moving dim>=256.
  - Softmax needs NO reductions in this layout: exp() is applied directly
    (fp32 dynamic range covers |scores| <= ~50 without max subtraction), the
    causal mask is applied by zeroing exp values (one 3D-AP affine_select
    covering all diagonal triangles of a group), and the denominator falls
    out of the PV matmul via a ones-column appended to V:
    out_aug[0:64, q] = numerator, out_aug[64, q] = denominator.
  - exp runs mostly on ScalarE; a tunable number of score groups instead use
    a 4-instruction VectorE exp (Schraudolph exponent construction + exact
    mantissa extraction + cubic correction, ~7e-4 max rel err) to balance
    the two engines.
  - Normalize: the denominator row is broadcast across 64 partitions with a
    K=1 matmul against a ones vector, then fast reciprocal and one multiply.
  - Per-head output is d-major ([64, 2048]); the host transposes it back
    during unsharding (pure layout, no math).
"""

import math
import numpy as np
from contextlib import ExitStack

import concourse.bass as bass
import concourse.bacc as bacc
import concourse.mybir as mybir
import concourse.tile as tile
from concourse.bass_utils import run_bass_kernel_spmd

B, H, L, D = 2, 16, 2048, 64
N_CORES = 8
HPC = (B * H) // N_CORES  # heads per core = 4

F32 = mybir.dt.float32
F32R = mybir.dt.float32r
I32 = mybir.dt.int32
EXP = mybir.ActivationFunctionType.Exp

# ---------------------------------------------------------------------------
# Custom DVE ops for the VectorE exp path.
#
# exp(x) = 2^y, y = x*log2(e).  i = int32(y*2^23 + 127*2^23) gives
# u = bitcast_f32(i) = 2^n * (1+m) with n+m = quantized y (exact in bits).
# True value = 2^(n+m) = u * c(w),  w = 1+m in [1,2),
# c(w) = 2^(w-1)/w, approximated by a cubic (max rel err 6.7e-4).
# w is recovered exactly from u's bits: (i & 0x7FFFFF) | 0x3F800000.
# ---------------------------------------------------------------------------
import concourse.dve_ops as dve_ops
from concourse.dve_spec import (
    AluOp,
    Bin,
    C0,
    C1,
    C2,
    C3,
    Spec,
    Src0,
    _spill_c3_to_src1,
    lower,
    _has_src1,
)
from concourse.dve_uop import DveOpSpec

_MANT_MASK_F = float(np.int32(0x007FFFFF).view(np.float32))  # denormal bits
_ONE_F = 1.0  # bits 0x3F800000

# cubic minimax fit of 2^(w-1)/w on [1,2], rel err <= 6.7e-4
_C3_COEF = (1.77561472, -1.37730759, 0.70747583, -0.1064457)
_SCHRAUD_A = float(np.float32(np.log2(math.e) * 2.0**23))
_SCHRAUD_B = float(np.float32(127.0 * 2.0**23))


def _ref_exp_w(in0, in1, s0, s1, imm2):
    bits = np.asarray(in0, np.float32).view(np.int32)
    w = (bits & np.int32(0x007FFFFF)) | np.int32(0x3F800000)
    return w.view(np.float32)


def _ref_exp_p3(in0, in1, s0, s1, imm2):
    # in1 carries c3 (C3 spilled to Src1 as a [P,1] scalar)
    return s0 + in0 * (s1 + in0 * (imm2 + in0 * in1))


def _make_op(name, body, reference):
    spec = Spec(body=body, reference=reference)
    shas = {}
    for ver in ("v3", "v4"):
        s = DveOpSpec(
            name=name,
            opcode=0,
            uops=lower(spec, ver=ver),
            rd1_en=_has_src1(spec),
        )
        shas[ver] = s.sha(ver)
    op = dve_ops.DveOp(name, spec, subdim=False, uops_sha=shas)
    if name not in dve_ops._SUB_OPCODE_FOR_NAME:
        row = max(dve_ops._SUB_OPCODE_FOR_NAME.values()) + 1
        assert row < 0x20
        dve_ops._SUB_OPCODE_FOR_NAME[name] = row
        dve_ops.OPS.append(op)
        dve_ops.CUSTOM_DVE_SPECS[name] = spec
    return op


EXP_W_ANT = _make_op(
    "EXP_W_ANT",
    Bin(AluOp.BITWISE_OR, Bin(AluOp.BITWISE_AND, Src0, C0), C1),
    _ref_exp_w,
)
EXP_P3_ANT = _make_op(
    "EXP_P3_ANT",
    _spill_c3_to_src1(C0 + Src0 * (C1 + Src0 * (C2 + Src0 * C3))),
    _ref_exp_p3,
)


def _j_order(nj):
    # measured-best chunk processing order (ends on a mid-size chunk)
    return [x for x in (0, 2, 3, 1) if x < nj] or list(range(nj))


def build_nc(hpc=HPC, seq=L, dim=D, qw=512, dve_g=0):
    """Build the single-core Bass/Tile program (shared SPMD across cores)."""
    assert seq % qw == 0 and qw % 128 == 0
    nj = seq // qw          # number of q chunks
    gkb = qw // 128         # k-blocks in the diagonal group
    nkb = seq // 128        # total k blocks
    assert hpc % 2 == 0
    npair = hpc // 2

    nc = bacc.Bacc(trn_type="TRN2")
    # head-PAIR packed q/k: pair p rows 0:64 = head 2p, rows 64:128 = head 2p+1
    qT = nc.dram_tensor("qT", [npair, 2 * dim, seq], F32R, kind="ExternalInput")
    kT = nc.dram_tensor("kT", [npair, 2 * dim, seq], F32R, kind="ExternalInput")
    vA = nc.dram_tensor("vA", [hpc, seq, dim + 1], F32R, kind="ExternalInput")
    onesIn = nc.dram_tensor("ones_in", [128, dim], F32R, kind="ExternalInput")
    oT = nc.dram_tensor("oT", [hpc, dim, seq], F32, kind="ExternalOutput")

    with tile.TileContext(nc) as tc, ExitStack() as ctx:
        qk_pool = ctx.enter_context(tc.tile_pool(name="qk", bufs=1))
        v_pool = ctx.enter_context(tc.tile_pool(name="v", bufs=1))
        exp_pool = ctx.enter_context(tc.tile_pool(name="exp", bufs=8))
        expb_pool = ctx.enter_context(tc.tile_pool(name="expb", bufs=5))
        misc_pool = ctx.enter_context(tc.tile_pool(name="misc", bufs=4))
        dve_pool = ctx.enter_context(tc.tile_pool(name="dve", bufs=2))
        const_pool = ctx.enter_context(tc.tile_pool(name="const", bufs=1))
        ps_a = ctx.enter_context(tc.tile_pool(name="ps_a", bufs=1, space="PSUM"))
        ps_b = ctx.enter_context(tc.tile_pool(name="ps_b", bufs=1, space="PSUM"))
        ps_o = ctx.enter_context(tc.tile_pool(name="ps_o", bufs=2, space="PSUM"))

        # --- load inputs; q/k arrive split per q-chunk, highest chunk first
        # (chunks are processed j-descending, and chunk j's diagonal group
        # needs only column block j) ---
        qts = [None] * npair
        kts = [None] * npair
        vas = [None] * hpc
        for p in range(npair):
            qts[p] = qk_pool.tile([2 * dim, seq], F32R, name=f"qt{p}", tag=f"qt{p}")
            kts[p] = qk_pool.tile([2 * dim, seq], F32R, name=f"kt{p}", tag=f"kt{p}")
        for p in range(npair):
            for c in range(nj):
                cs = slice(c * qw, (c + 1) * qw)
                nc.sync.dma_start(kts[p][:, cs], kT[p][:, cs])
                nc.sync.dma_start(qts[p][:, cs], qT[p][:, cs])
        for h in range(hpc):
            vas[h] = v_pool.tile(
                [128, nkb, dim + 1], F32R, name=f"va{h}", tag=f"va{h}"
            )
            nc.sync.dma_start(
                vas[h][:], vA[h].rearrange("(ko ki) d -> ki ko d", ki=128)
            )

        c3ap = const_pool.tile([128, 1], F32, name="c3ap", tag="c3ap")
        nc.vector.memset(c3ap[:], _C3_COEF[3])
        ones = const_pool.tile([128, dim], F32R, name="ones", tag="ones")
        nc.sync.dma_start(ones[:], onesIn[:])

        def dve_exp(ps_slice, et_slice, w):
            """VectorE exp: ps_slice [128, w] (PSUM f32) -> et_slice (f32r)."""
            t_i = dve_pool.tile([128, 3 * qw], I32, name="dve_i", tag="dve_i")[
                :, :w
            ]
            nc.vector.tensor_scalar(
                t_i, ps_slice, _SCHRAUD_A, _SCHRAUD_B,
                mybir.AluOpType.mult, mybir.AluOpType.add,
            )
            u_f = t_i.bitcast(F32)
            w_f = dve_pool.tile([128, 3 * qw], F32, name="dve_w", tag="dve_w")[
                :, :w
            ]
            nc.vector._custom_dve(
                EXP_W_ANT, out=w_f, in0=u_f, s0=_MANT_MASK_F, s1=_ONE_F
            )
            p_f = dve_pool.tile([128, 3 * qw], F32, name="dve_p", tag="dve_p")[
                :, :w
            ]
            nc.vector._custom_dve(
                EXP_P3_ANT,
                out=p_f,
                in0=w_f,
                in1=c3ap[:],
                s0=_C3_COEF[0],
                s1=_C3_COEF[1],
                imm2=_C3_COEF[2],
            )
            nc.vector.tensor_mul(et_slice, u_f, p_f)

        # --- main loop. Heads are processed in PAIRS: the two heads of a
        # pair live on SBUF partitions 0:64 / 64:128, so their score
        # matmuls (contraction dim 64) target disjoint PE row-groups and
        # run CONCURRENTLY on hardware (tile_position auto-derives from
        # base_partition). Score PSUM tiles are shared by the pair:
        #   A tile [128, 2048] = 2 k-blocks x 2 heads x 512 q
        #   B tile [128, 1024] = 1 k-block  x 2 heads x 512 q
        # A(4 banks) + B(2) + PV out x2 (2) = 8 PSUM banks. ---
        for j in _j_order(nj):
            for p in range(npair):
                nblk = gkb * (j + 1)
                qhs = [qts[p][r : r + dim, :] for r in (0, dim)]
                khs = [kts[p][r : r + dim, :] for r in (0, dim)]
                # plan: list of (tag, [kb...], diag) — nondiag k-blocks in
                # alternating A(2 kbs)/B(1 kb) groups, diag k-blocks last
                # as A(2)+A(2).
                plans = []
                s = 0
                gi = 0
                nd = gkb * j
                while s < nd:
                    cnt = 2 if gi % 2 == 0 else 1
                    cnt = min(cnt, nd - s)
                    plans.append(
                        ("A" if cnt == 2 else "B", list(range(s, s + cnt)), False)
                    )
                    s += cnt
                    gi += 1
                for g in range(gkb // 2):
                    plans.append(
                        ("A", [nd + 2 * g, nd + 2 * g + 1], True)
                    )

                # exp_slices[h][kb] = (rhs_slice, off)
                exp_slices = [[None] * nblk for _ in range(2)]
                for tag, kbs, diag in plans:
                    ncol = 2 * len(kbs)  # 512-wide columns used
                    if tag == "A":
                        ps = ps_a.tile([128, 4 * qw], F32, name="psA", tag="psA")
                        et = exp_pool.tile(
                            [128, 4 * qw], F32R, name="etA", tag="etA"
                        )
                    else:
                        ps = ps_b.tile([128, 2 * qw], F32, name="psB", tag="psB")
                        et = expb_pool.tile(
                            [128, 2 * qw], F32R, name="etB", tag="etB"
                        )
                    # score matmuls: heads interleaved so consecutive MMs hit
                    # disjoint PE row groups (base partition 0 vs 64).
                    # Diagonal blocks with off >= 256 are causally TRIMMED and
                    # written at their bank base (one matmul per PSUM bank,
                    # bank-aligned start - both HW rules respected); their exp
                    # then reads only the valid columns via a strided AP.
                    trim = diag and (kbs[0] - gkb * j) * 128 >= 256
                    for u, kb in enumerate(kbs):
                        off = (kb - gkb * j) * 128 if trim else 0
                        for hh in range(2):
                            col = 2 * u + hh
                            nc.tensor.matmul(
                                ps[:, col * qw : (col + 1) * qw - off],
                                lhsT=khs[hh][:, kb * 128 : (kb + 1) * 128],
                                rhs=qhs[hh][:, j * qw + off : (j + 1) * qw],
                                start=True,
                                stop=True,
                            )
                    if trim:
                        # offs are (256,256,384,384) -> valid widths
                        # (256,256,128,128): one strided exp per width pair
                        pp = 4 * qw  # psum/et partition pitch (A tile)
                        for base_col, wv in ((0, qw - 256), (2, qw - 384)):
                            pin = bass.AP(
                                ps.tensor,
                                ps.offset + base_col * qw,
                                [[pp, 128], [qw, 2], [1, wv]],
                            )
                            pout = bass.AP(
                                et.tensor,
                                et.offset + base_col * qw,
                                [[pp, 128], [qw, 2], [1, wv]],
                            )
                            nc.scalar.activation(pout, pin, EXP)
                    else:
                        w = ncol * qw
                        nc.scalar.activation(et[:, :w], ps[:, :w], EXP)
                    for u, kb in enumerate(kbs):
                        off = (kb - gkb * j) * 128 if diag else 0
                        for hh in range(2):
                            col = 2 * u + hh
                            if trim:
                                exp_slices[hh][kb] = (
                                    et[:, col * qw : (col + 1) * qw - off],
                                    off,
                                )
                            else:
                                exp_slices[hh][kb] = (
                                    et[:, col * qw + off : (col + 1) * qw],
                                    off,
                                )
                    if diag:
                        for u, kb in enumerate(kbs):
                            off = (kb - gkb * j) * 128
                            tb = 0 if trim else off
                            for hh in range(2):
                                col = 2 * u + hh
                                sl = et[:, col * qw + tb : col * qw + tb + 128]
                                nc.gpsimd.affine_select(
                                    out=sl,
                                    in_=sl,
                                    compare_op=mybir.AluOpType.is_ge,
                                    fill=0.0,
                                    base=0,
                                    pattern=[[1, 128]],
                                    channel_multiplier=-1,
                                )

                # PV + normalize per head
                for hh in range(2):
                    h = 2 * p + hh
                    po = ps_o.tile([dim + 1, qw], F32, name="ps_o", tag="ps_o")
                    for i, kb in enumerate(range(nblk)):
                        rhs, off = exp_slices[hh][kb]
                        nc.tensor.matmul(
                            po[:, off:],
                            lhsT=vas[h][:, kb, :],
                            rhs=rhs,
                            start=(i == 0),
                            stop=(i == nblk - 1),
                        )
                    oa = misc_pool.tile([dim + 1, qw], F32R, name="oa", tag="oa")
                    nc.vector.tensor_copy(oa[:], po[:])
                    rec = misc_pool.tile([dim, qw], F32, name="rec", tag="rec")
                    pb = ps_o.tile([dim, qw], F32, name="ps_o", tag="ps_o")
                    nc.tensor.matmul(
                        pb[:],
                        lhsT=ones[dim : dim + 1, :],
                        rhs=oa[dim : dim + 1, :],
                        start=True,
                        stop=True,
                    )
                    nc.vector.reciprocal_approx_fast(rec[:], pb[:])
                    ob = misc_pool.tile([dim, qw], F32, name="ob", tag="ob")
                    nc.vector.tensor_mul(ob[:], oa.bitcast(F32)[:dim, :], rec[:])
                    nc.sync.dma_start(oT[h][:, j * qw : (j + 1) * qw], ob[:])
    nc.compile()
    return nc


_NC_CACHE = {}


def _get_nc(key=(HPC, L, D, 512)):
    if key not in _NC_CACHE:
        _NC_CACHE[key] = build_nc(*key)
    return _NC_CACHE[key]


def make_in_maps(q, k, v):
    """Shard + lay out the full [B,H,L,D] inputs into per-core device maps."""
    qf = np.ascontiguousarray(q, dtype=np.float32).reshape(B * H, L, D)
    kf = np.ascontiguousarray(k, dtype=np.float32).reshape(B * H, L, D)
    vf = np.ascontiguousarray(v, dtype=np.float32).reshape(B * H, L, D)
    in_maps = []
    for c in range(N_CORES):
        sl = slice(HPC * c, HPC * (c + 1))
        # [hpc, L, D] -> [hpc, D, L] -> head-pair packed [hpc//2, 2D, L]
        qTc = np.ascontiguousarray(qf[sl].transpose(0, 2, 1)).reshape(
            HPC // 2, 2 * D, L
        )
        kTc = np.ascontiguousarray(kf[sl].transpose(0, 2, 1)).reshape(
            HPC // 2, 2 * D, L
        )
        vAc = np.concatenate(
            [vf[sl], np.ones((HPC, L, 1), dtype=np.float32)], axis=2
        )
        in_maps.append(
            {
                "qT": qTc,
                "kT": kTc,
                "vA": np.ascontiguousarray(vAc),
                "ones_in": np.ones((128, D), dtype=np.float32),
            }
        )
    return in_maps


def gather_output(results):
    """Per-core oT [hpc, D, L] -> full [B, H, L, D]."""
    oT = np.concatenate([r["oT"] for r in results], axis=0)  # [B*H, D, L]
    return np.ascontiguousarray(
        oT.transpose(0, 2, 1).reshape(B, H, L, D).astype(np.float32)
    )


def run(q, k, v, trace=False, **spmd_kwargs):
    nc = _get_nc()
    res = run_bass_kernel_spmd(
        nc,
        make_in_maps(q, k, v),
        core_ids=list(range(N_CORES)),
        trace=trace,
        **spmd_kwargs,
    )
    return gather_output(res.results), res


def kernel(q, k, v):
    out, _ = run(q, k, v)
    return out

